# revision 1
# baseline (speedup 1.0000x reference)
"""Trainium2 Bass kernel for nn_ODE_71743133713072.

Semantics (unrolled from the reference lax.scan):
  out[:, 0]   = lat[:, 0]
  out[:, t+1] = lat[:, t] + dt_eff[t] * f(lat[:, t])   for t = 0..99
                (dt_eff[1] = 0 reproduces the scan's zero-length first gap)
  y = out[:, 100]
  out[:, k+1] = y = y + h * f(y)                        for k = 100..118
where f is the D->U->U->D tanh MLP and all nonzero dt equal h = ts[1]-ts[0]
(linspace; per-step fp32 diffs differ from h by <=1 ulp, far below the bf16
matmul noise floor, so h is folded into W3/b3 on the host).

Sharding: batch 1024 over 8 cores (128 rows/core, exactly the partition
width). Matmuls in bf16 with fp32 PSUM accumulation. Layers 1-2 run
feature-on-partition (weights stationary); layer 3 swaps roles (activations
stationary, h*W3 moving) so its output lands in natural row layout and the
Euler update is a single PSUM+SBUF add. b3*h enters layer 3's accumulation
group via a K=1 ones-row matmul.
"""

import os
import sys
from contextlib import ExitStack

import numpy as np

for _p in ("/opt/trn_rl_repo", "/root/.axon_site/_ro/trn_rl_repo"):
    if os.path.isdir(_p) and _p not in sys.path:
        sys.path.append(_p)

import ml_dtypes  # noqa: E402

B, T_OBS, KPRED, D = 1024, 100, 20, 256
T = T_OBS + KPRED          # 120
NCORES = 8
PB = B // NCORES           # 128 rows per core
P = 128
G = 4                      # time steps per compute group
NG = T_OBS // G            # 25 groups


def _emit(ctx, tc, lat, w3hd, w8d, bpk, br8p, id8d, id32d, ones8d, out, h):
    import concourse.mybir as mybir

    nc = tc.nc
    F32 = mybir.dt.float32
    BF16 = mybir.dt.bfloat16
    FP8 = mybir.dt.float8e4
    AF = mybir.ActivationFunctionType
    ALU = mybir.AluOpType
    DR = mybir.MatmulPerfMode.DoubleRow

    const = ctx.enter_context(tc.tile_pool(name="const", bufs=1))
    w3sb = const.tile([P, 2, D], BF16, tag="w3")
    for kc in range(2):
        nc.sync.dma_start(w3sb[:, kc, :], w3hd[kc * P:(kc + 1) * P, :])
    bsb = const.tile([P, 6], F32, tag="bias")
    nc.sync.dma_start(bsb[:], bpk[:])
    ones8 = const.tile([1, P], FP8, tag="ones8")
    nc.sync.dma_start(ones8[:], ones8d[:])
    # fp8 weights (x8-scaled): [P, 3(w), 2(kc), D]
    w8sb = const.tile([P, 3, 2, D], FP8, tag="w8")
    for wi in range(3):
        for kc in range(2):
            nc.sync.dma_start(w8sb[:, wi, kc, :], w8d[wi, kc * P:(kc + 1) * P, :])
    br8sb = const.tile([1, 2 * D], FP8, tag="br8")
    nc.sync.dma_start(br8sb[:], br8p[:])
    id16 = const.tile([P, P], BF16, tag="id16")
    nc.sync.dma_start(id16[:], id8d[:])
    id32 = const.tile([P, P], F32, tag="id32")
    nc.sync.dma_start(id32[:], id32d[:])

    b1ap = [bsb[:, 0:1], bsb[:, 1:2]]
    b2ap = [bsb[:, 2:3], bsb[:, 3:4]]
    b3hap = [bsb[:, 4:5], bsb[:, 5:6]]

    x32p = ctx.enter_context(tc.tile_pool(name="x32", bufs=4))
    x8p = ctx.enter_context(tc.tile_pool(name="x8", bufs=4))
    xtsbp = ctx.enter_context(tc.tile_pool(name="xtsb", bufs=3))
    hsbp = ctx.enter_context(tc.tile_pool(name="hsb", bufs=4))
    outp = ctx.enter_context(tc.tile_pool(name="outsb", bufs=4))
    chsb = ctx.enter_context(tc.tile_pool(name="chsb", bufs=3))

    xtps = ctx.enter_context(tc.tile_pool(name="xtps", bufs=1, space="PSUM"))
    mmps = ctx.enter_context(tc.tile_pool(name="mmps", bufs=2, space="PSUM"))
    fnps = ctx.enter_context(tc.tile_pool(name="fnps", bufs=2, space="PSUM"))
    chps = ctx.enter_context(tc.tile_pool(name="chps", bufs=1, space="PSUM"))

    h8 = float(h / 8.0)

    def stage_load(g):
        """load + cast + transpose + evac for one group; returns tiles."""
        t0 = g * G
        x32 = x32p.tile([P, G, D], F32, tag="x32")
        nc.sync.dma_start(x32[:], lat[:, t0:t0 + G, :])
        x16 = x8p.tile([P, G, D], BF16, tag="x16")
        nc.gpsimd.tensor_copy(x16[:], x32[:])
        xt = xtps.tile([P, 2, G * P], BF16, tag="xt")
        for tt in range(G):
            for dc in range(2):
                nc.tensor.transpose(
                    xt[:, dc, tt * P:(tt + 1) * P],
                    x16[:, tt, dc * P:(dc + 1) * P], id16[:])
        xts = xtsbp.tile([P, 2, G * P], FP8, tag="xts")
        for dc in range(2):
            nc.vector.tensor_copy(xts[:, dc, :], xt[:, dc, :])
        return x32, xts

    def stage_mlp(pair):
        """L1/L2 for a pair of groups with shared weight loads."""
        h1s = {}
        mm = {}
        for g, (x32, xts) in pair.items():
            mm[g] = mmps.tile([P, 2, G * P], F32, tag="mm", name="mm")
        for mc in range(2):
            for g in pair:
                nc.tensor.matmul(mm[g][:, mc, :],
                                 w8sb[:, 0, :, mc * P:(mc + 1) * P],
                                 pair[g][1][:], start=True, stop=True,
                                 perf_mode=DR)
        for g in pair:
            t = hsbp.tile([P, 2, G * P], FP8, tag="h1")
            for mc in range(2):
                nc.scalar.activation(t[:, mc, :], mm[g][:, mc, :], AF.Tanh,
                                     bias=b1ap[mc], scale=0.125)
            h1s[g] = t
        mm2 = {}
        for g in pair:
            mm2[g] = mmps.tile([P, 2, G * P], F32, tag="mm", name="mm2")
        for mc in range(2):
            for g in pair:
                nc.tensor.matmul(mm2[g][:, mc, :],
                                 w8sb[:, 1, :, mc * P:(mc + 1) * P],
                                 h1s[g][:], start=True, stop=True,
                                 perf_mode=DR)
        h2s = {}
        for g in pair:
            t = hsbp.tile([P, 2, G * P], FP8, tag="h2")
            for mc in range(2):
                nc.scalar.activation(t[:, mc, :], mm2[g][:, mc, :], AF.Tanh,
                                     bias=b2ap[mc], scale=0.125)
            h2s[g] = t
        return h2s

    def stage_out(g, x32, h2s_g):
        """L3 (role-swapped, fp8 DR) + Euler add + store for one group."""
        t0 = g * G
        o32 = outp.tile([P, G, D], F32, tag="o32")
        for half in range(2):
            fn = fnps.tile([P, 2, D], F32, tag="fn")
            # seed each subtile with 8*b3 broadcast (K=1 ones row)
            for i, tt in enumerate((2 * half, 2 * half + 1)):
                nc.tensor.matmul(fn[:, i, :], ones8[:], br8sb[:, 0:D],
                                 start=True, stop=False)
                nc.tensor.matmul(fn[:, i, :],
                                 h2s_g[:, :, tt * P:(tt + 1) * P],
                                 w8sb[:, 2, :, :],
                                 start=False, stop=True, perf_mode=DR)
            if g == 0 and half == 0:
                # t=0: normal Euler step; t=1: dt=0 -> out[:,2] = lat[:,1]
                nc.vector.scalar_tensor_tensor(
                    o32[:, 0, :], fn[:, 0, :], h8, x32[:, 0, :],
                    ALU.mult, ALU.add)
                nc.vector.tensor_copy(o32[:, 1, :], x32[:, 1, :])
            else:
                nc.vector.scalar_tensor_tensor(
                    o32[:, 2 * half:2 * half + 2, :].rearrange("p a b -> p (a b)"),
                    fn.rearrange("p a b -> p (a b)"), h8,
                    x32[:, 2 * half:2 * half + 2, :].rearrange("p a b -> p (a b)"),
                    ALU.mult, ALU.add)
        nc.sync.dma_start(out[:, t0 + 1:t0 + G + 1, :], o32[:])
        return o32

    def do_pair(ga, gb):
        pair = {}
        for g in (ga, gb):
            if g is not None:
                pair[g] = stage_load(g)
        h2s = stage_mlp(pair)
        outs = {}
        for g in pair:
            outs[g] = stage_out(g, pair[g][0], h2s[g])
        return outs

    def chain(o32_24):
        # y0 = out[:, 100] = o32_24[:, 3, :]; chain state transposed fp32.
        y0p = chps.tile([P, 2, P], F32, tag="ch")
        for dc in range(2):
            nc.tensor.transpose(y0p[:, dc, :],
                                o32_24[:, G - 1, dc * P:(dc + 1) * P], id32[:])
        yt = chsb.tile([P, 2, P], F32, tag="yt")
        nc.vector.tensor_copy(yt[:], y0p[:])

        for k in range(T_OBS, T - 1):
            y8 = chsb.tile([P, 2, P], FP8, tag="y8")
            nc.vector.tensor_copy(y8[:], yt[:])
            c1 = chps.tile([P, 2, P], F32, tag="ch")
            for mc in range(2):
                nc.tensor.matmul(c1[:, mc, :],
                                 w8sb[:, 0, :, mc * P:(mc + 1) * P],
                                 y8[:], start=True, stop=True, perf_mode=DR)
            c1s = chsb.tile([P, 2, P], FP8, tag="c1s")
            for mc in range(2):
                nc.scalar.activation(c1s[:, mc, :], c1[:, mc, :], AF.Tanh,
                                     bias=b1ap[mc], scale=0.125)
            c2 = chps.tile([P, 2, P], F32, tag="ch")
            for mc in range(2):
                nc.tensor.matmul(c2[:, mc, :],
                                 w8sb[:, 1, :, mc * P:(mc + 1) * P],
                                 c1s[:], start=True, stop=True, perf_mode=DR)
            c2s = chsb.tile([P, 2, P], BF16, tag="c2s")
            for mc in range(2):
                nc.scalar.activation(c2s[:, mc, :], c2[:, mc, :], AF.Tanh,
                                     bias=b2ap[mc], scale=0.125)
            # L3 in bf16 (w3sb = h*W3); b3*h joins in the update op below.
            c3 = chps.tile([P, 2, P], F32, tag="ch")
            for mc in range(2):
                for kc in range(2):
                    nc.tensor.matmul(c3[:, mc, :],
                                     w3sb[:, kc, mc * P:(mc + 1) * P],
                                     c2s[:, kc, :], start=(kc == 0),
                                     stop=(kc == 1))
            ytn = chsb.tile([P, 2, P], F32, tag="yt")
            for dc in range(2):
                nc.vector.scalar_tensor_tensor(
                    ytn[:, dc, :], c3[:, dc, :], b3hap[dc], yt[:, dc, :],
                    ALU.add, ALU.add)
            yt = ytn

            ynp = chps.tile([P, D], F32, tag="ch")
            for dc in range(2):
                nc.tensor.transpose(ynp[:, dc * P:(dc + 1) * P], yt[:, dc, :], id32[:])
            yns = chsb.tile([P, D], F32, tag="yns")
            nc.vector.tensor_copy(yns[:], ynp[:])
            nc.sync.dma_start(out[:, k + 1, :], yns[:])

    outs = do_pair(NG - 1, NG - 2)
    chain(outs[NG - 1])
    for p in range(0, NG - 2, 2):
        ga = p
        gb = p + 1 if p + 1 < NG - 2 else None
        do_pair(ga, gb)
    nc.sync.dma_start(out[:, 0, :], lat[:, 0, :])


def _build(h):
    import concourse.mybir as mybir
    import concourse.tile as tile
    from concourse import bacc

    F32 = mybir.dt.float32
    BF16 = mybir.dt.bfloat16
    FP8 = mybir.dt.float8e4

    nc = bacc.Bacc("TRN2", target_bir_lowering=False, debug=False,
                   num_devices=NCORES)
    lat = nc.dram_tensor("lat", [PB, T_OBS, D], F32, kind="ExternalInput").ap()
    w3hd = nc.dram_tensor("w3h", [D, D], BF16, kind="ExternalInput").ap()
    w8d = nc.dram_tensor("w8", [3, D, D], FP8, kind="ExternalInput").ap()
    bpk = nc.dram_tensor("bpack", [P, 6], F32, kind="ExternalInput").ap()
    br8p = nc.dram_tensor("brows8", [1, 2 * D], FP8, kind="ExternalInput").ap()
    id8d = nc.dram_tensor("id8", [P, P], BF16, kind="ExternalInput").ap()
    id32d = nc.dram_tensor("id32", [P, P], F32, kind="ExternalInput").ap()
    ones8d = nc.dram_tensor("ones8", [1, P], FP8, kind="ExternalInput").ap()
    out = nc.dram_tensor("out", [PB, T, D], F32, kind="ExternalOutput").ap()

    with tile.TileContext(nc) as tc, ExitStack() as ctx:
        _emit(ctx, tc, lat, w3hd, w8d, bpk, br8p, id8d, id32d, ones8d, out, h)
    nc.compile()
    return nc


def _host_inputs(inputs):
    ts = np.asarray(inputs["time_steps"], np.float32)
    h = float(np.float32(ts[1]) - np.float32(ts[0]))

    bf = ml_dtypes.bfloat16
    f8 = ml_dtypes.float8_e4m3
    w3h = (np.asarray(inputs["W3"], np.float32) * np.float32(h)).astype(bf)
    b1 = np.asarray(inputs["b1"], np.float32)
    b2 = np.asarray(inputs["b2"], np.float32)
    b3h = np.asarray(inputs["b3"], np.float32) * np.float32(h)
    bpack = np.stack([b1[:P], b1[P:], b2[:P], b2[P:], b3h[:P], b3h[P:]],
                     axis=1).astype(np.float32)
    w8 = np.stack([
        (8.0 * np.asarray(inputs["W1"], np.float32)),
        (8.0 * np.asarray(inputs["W2"], np.float32)),
        (8.0 * np.asarray(inputs["W3"], np.float32)),
    ]).astype(f8)
    b3s8 = (8.0 * np.asarray(inputs["b3"], np.float32))
    brows8 = np.concatenate([b3s8, b3s8]).reshape(1, 2 * D).astype(f8)
    id8 = np.eye(P, dtype=np.float32).astype(bf)
    id32 = np.eye(P, dtype=np.float32)
    ones8 = np.ones((1, P), np.float32).astype(f8)

    shared = dict(w3h=w3h, w8=w8, bpack=bpack, brows8=brows8,
                  id8=id8, id32=id32, ones8=ones8)
    return h, shared


_CACHE = {}


def kernel(**inputs):
    from concourse.bass_utils import run_bass_kernel_spmd

    lat_full = np.ascontiguousarray(np.asarray(inputs["latents"], np.float32))
    h, shared = _host_inputs(inputs)

    if h not in _CACHE:
        _CACHE[h] = _build(h)
    nc = _CACHE[h]

    in_maps = []
    for c in range(NCORES):
        m = dict(shared)
        m["lat"] = np.ascontiguousarray(lat_full[c * PB:(c + 1) * PB])
        in_maps.append(m)
    res = run_bass_kernel_spmd(nc, in_maps, list(range(NCORES)))
    outs = [res.results[c]["out"] for c in range(NCORES)]
    return np.concatenate(outs, axis=0)



# revision 6
# speedup vs baseline: 1.7187x; 1.7187x over previous
"""Trainium2 Bass kernel for nn_ODE_71743133713072 (v2).

Semantics (unrolled from the reference lax.scan):
  out[:, 0]   = lat[:, 0]                       (host)
  out[:, 2]   = lat[:, 1]                       (host; the scan's dt=0 step)
  out[:, t+1] = lat[:, t] + h*f(lat[:, t])      t = 0..99   (parallel part)
  out[:, k+1] = y += h*f(y), y0 = out[:, 100]   k = 100..118 (serial chain)
with f the D->U->U->D tanh MLP, h = ts[1]-ts[0].

Key design points (all validated on hw in micro benchmarks):
  - Batch 1024 split over 8 cores (128 rows = partition width per core).
  - x^T for layer 1 arrives via hardware DMA-transpose of an fp8 copy of
    the latents packed as uint16 byte-pairs; DoubleRow fp8 matmuls consume
    the pairs through a byte-interleaved access pattern (k = 2p+j) with
    host-permuted W1 rows. No on-chip transpose/cast of x at all.
  - h*b3 is folded into the natural-layout euler operand on the host
    (latb = bf16(lat + h*b3)), so layer 3 needs no bias seed matmul.
  - b1/b2 activation biases use a pairing permutation: U features are
    sorted by bias so the two features sharing an SBUF partition have
    nearly equal bias; one [p,1] bias AP then covers a whole layer ->
    a single free-1024 activation instruction per layer per group.
  - Outputs are stored bf16 (t=1..100) / f32-transposed (chain) and
    assembled to f32 on the host; rel-err budget is ~2e-3 vs 2e-2 gate.
  - DMA issue is split between the Sync HWDGE queue (transposed loads,
    bf16 stores) and the GpSimd SWDGE queue (natural loads, chain stores).
"""

import os
import sys
from contextlib import ExitStack

import numpy as np

for _p in ("/opt/trn_rl_repo", "/root/.axon_site/_ro/trn_rl_repo"):
    if os.path.isdir(_p) and _p not in sys.path:
        sys.path.append(_p)

import ml_dtypes  # noqa: E402

B, T_OBS, KPRED, D = 1024, 100, 20, 256
T = T_OBS + KPRED          # 120
NCORES = 8
PB = B // NCORES           # 128 rows per core
P = 128
G = 4                      # time steps per compute group
NG = T_OBS // G            # 25 groups


def _emit(ctx, tc, t_lat8u, t_latb, t_w1i, t_w2i, t_w3m, t_w1c, t_w3c,
          t_hb3, t_ones, t_bs, t_id32, t_out16, t_outch, h):
    import concourse.mybir as mybir

    nc = tc.nc
    F32 = mybir.dt.float32
    BF16 = mybir.dt.bfloat16
    FP8 = mybir.dt.float8e4
    AF = mybir.ActivationFunctionType
    ALU = mybir.AluOpType
    DR = mybir.MatmulPerfMode.DoubleRow

    h8 = float(h / 8.0)

    const = ctx.enter_context(tc.tile_pool(name="const", bufs=1))
    w1i = const.tile([P, 2, 2 * P], FP8, tag="w1i")
    nc.sync.dma_start(w1i[:], t_w1i[:])
    w2i = const.tile([P, 2, 2 * P], FP8, tag="w2i")
    nc.sync.dma_start(w2i[:], t_w2i[:])
    w3m = const.tile([P, 2, 2 * P], FP8, tag="w3m")
    nc.sync.dma_start(w3m[:], t_w3m[:])
    w1c = const.tile([P, 2, 2 * P], FP8, tag="w1c")
    nc.sync.dma_start(w1c[:], t_w1c[:])
    w3c = const.tile([P, 2, 2 * P], BF16, tag="w3c")
    nc.sync.dma_start(w3c[:], t_w3c[:])
    hb3 = const.tile([1, 2 * P], BF16, tag="hb3")
    nc.sync.dma_start(hb3[:], t_hb3[:])
    ones = const.tile([1, P], BF16, tag="ones")
    nc.sync.dma_start(ones[:], t_ones[:])
    bs = const.tile([P, 2], F32, tag="bs")
    nc.sync.dma_start(bs[:], t_bs[:])
    id32 = const.tile([P, P], F32, tag="id32")
    nc.sync.dma_start(id32[:], t_id32[:])
    chainbuf = const.tile([P, KPRED - 1, 2, P], F32, tag="chainbuf")

    b1s = bs[:, 0:1]
    b2s = bs[:, 1:2]

    xtsp = ctx.enter_context(tc.tile_pool(name="xts", bufs=6))
    x16p = ctx.enter_context(tc.tile_pool(name="x16", bufs=6))
    h1p = ctx.enter_context(tc.tile_pool(name="h1", bufs=2))
    h2p = ctx.enter_context(tc.tile_pool(name="h2", bufs=2))
    o16p = ctx.enter_context(tc.tile_pool(name="o16", bufs=3))
    chsb = ctx.enter_context(tc.tile_pool(name="chsb", bufs=4))

    mmp = ctx.enter_context(tc.tile_pool(name="mmp", bufs=2, space="PSUM"))
    fnp = ctx.enter_context(tc.tile_pool(name="fnp", bufs=2, space="PSUM"))
    chp = ctx.enter_context(tc.tile_pool(name="chp", bufs=2, space="PSUM"))

    def stage_load(g):
        t0 = g * G
        xts = xtsp.tile([P, G, P], BF16, tag="xts")
        nc.sync.dma_start_transpose(
            xts[:], t_lat8u[:, t0:t0 + G, :].rearrange("p a b -> p (a b)"))
        x16 = x16p.tile([P, G, 2 * P], BF16, tag="x16")
        nc.gpsimd.dma_start(x16[:], t_latb[:, t0:t0 + G, :])
        return xts, x16

    def stage_l1(xts):
        # interleaved fp8 view: [p, j, (t b)], k = 2p + j
        rhs1 = xts[:].bitcast(FP8).rearrange("p t (b j) -> p j (t b)", j=2)
        mm = mmp.tile([P, 2, G * P], F32, tag="mm", name="l1")
        for mc in range(2):
            nc.tensor.matmul(mm[:, mc, :], w1i[:, :, mc * P:(mc + 1) * P],
                             rhs1, start=True, stop=True, perf_mode=DR)
        return mm

    def stage_h1(mm):
        h1 = h1p.tile([P, 2, G * P], FP8, tag="h1")
        nc.scalar.activation(h1[:].rearrange("p a b -> p (a b)"),
                             mm[:].rearrange("p a b -> p (a b)"),
                             AF.Tanh, bias=b1s, scale=0.125)
        return h1

    def stage_l2(h1):
        mm2 = mmp.tile([P, 2, G * P], F32, tag="mm", name="l2")
        for mc in range(2):
            nc.tensor.matmul(mm2[:, mc, :], w2i[:, :, mc * P:(mc + 1) * P],
                             h1[:], start=True, stop=True, perf_mode=DR)
        return mm2

    def stage_h2(mm2):
        h2 = h2p.tile([P, 2, G * P], FP8, tag="h2")
        nc.scalar.activation(h2[:].rearrange("p a b -> p (a b)"),
                             mm2[:].rearrange("p a b -> p (a b)"),
                             AF.Tanh, bias=b2s, scale=0.125)
        return h2

    def stage_out(g, h2, x16, want_o32=False):
        t0 = g * G
        o16 = o16p.tile([P, G, 2 * P], BF16, tag="o16")
        o32 = None
        for half in range(2):
            fn = fnp.tile([P, 2, 2 * P], F32, tag="fn")
            for i in range(2):
                tt = 2 * half + i
                nc.tensor.matmul(fn[:, i, :], h2[:, :, tt * P:(tt + 1) * P],
                                 w3m[:], start=True, stop=True, perf_mode=DR)
            nc.vector.scalar_tensor_tensor(
                o16[:, 2 * half:2 * half + 2, :].rearrange("p a b -> p (a b)"),
                fn[:].rearrange("p a b -> p (a b)"), h8,
                x16[:, 2 * half:2 * half + 2, :].rearrange("p a b -> p (a b)"),
                ALU.mult, ALU.add)
            if want_o32 and half == 1:
                o32 = chsb.tile([P, 2, 2 * P], F32, tag="o32")
                nc.vector.scalar_tensor_tensor(
                    o32[:].rearrange("p a b -> p (a b)"),
                    fn[:].rearrange("p a b -> p (a b)"), h8,
                    x16[:, 2:4, :].rearrange("p a b -> p (a b)"),
                    ALU.mult, ALU.add)
        nc.sync.dma_start(t_out16[:, t0:t0 + G, :], o16[:])
        return o32

    def chain_init(o32):
        # y0 = out[:, 100] = o32[:, 1, :]; carry is y^T f32 [p(d), dc, b]
        y0p = chp.tile([P, 2, P], F32, tag="ch", name="y0p")
        for dc in range(2):
            nc.tensor.transpose(y0p[:, dc, :],
                                o32[:, 1, dc * P:(dc + 1) * P], id32[:])
        yt = chsb.tile([P, 2, P], F32, tag="yt")
        nc.vector.tensor_copy(yt[:], y0p[:])
        return yt

    def chain_step(k, yt):
        y8 = chsb.tile([P, 2, P], FP8, tag="y8")
        nc.vector.tensor_copy(y8[:], yt[:])
        c1 = chp.tile([P, 2, P], F32, tag="ch", name="c1")
        for mc in range(2):
            nc.tensor.matmul(c1[:, mc, :], w1c[:, :, mc * P:(mc + 1) * P],
                             y8[:], start=True, stop=True, perf_mode=DR)
        c1s = chsb.tile([P, 2, P], FP8, tag="c1s")
        nc.scalar.activation(c1s[:].rearrange("p a b -> p (a b)"),
                             c1[:].rearrange("p a b -> p (a b)"),
                             AF.Tanh, bias=b1s, scale=0.125)
        c2 = chp.tile([P, 2, P], F32, tag="ch", name="c2")
        for mc in range(2):
            nc.tensor.matmul(c2[:, mc, :], w2i[:, :, mc * P:(mc + 1) * P],
                             c1s[:], start=True, stop=True, perf_mode=DR)
        c2s = chsb.tile([P, 2, P], BF16, tag="c2s")
        nc.scalar.activation(c2s[:].rearrange("p a b -> p (a b)"),
                             c2[:].rearrange("p a b -> p (a b)"),
                             AF.Tanh, bias=b2s, scale=0.125)
        c3 = chp.tile([P, 2, P], F32, tag="ch", name="c3")
        for mc in range(2):
            # seed with h*b3 (rank-1), then accumulate h*W3^T c2s
            nc.tensor.matmul(c3[:, mc, :], hb3[:, mc * P:(mc + 1) * P],
                             ones[:], start=True, stop=False)
            for kc in range(2):
                nc.tensor.matmul(c3[:, mc, :],
                                 w3c[:, kc, mc * P:(mc + 1) * P],
                                 c2s[:, kc, :], start=False, stop=(kc == 1))
        ytn = chainbuf[:, k, :, :]
        nc.vector.tensor_tensor(ytn.rearrange("p a b -> p (a b)"),
                                c3[:].rearrange("p a b -> p (a b)"),
                                yt[:].rearrange("p a b -> p (a b)"), ALU.add)
        return ytn

    # --- emission ---
    # group 24 first (it feeds the chain), then 12 pairs of the remaining
    # 24 groups with stage interleaving (fills the in-order Act queue's
    # l2-wait gap with the other group's activation), with 1-2 chain steps
    # woven into each pair slot.
    xts24, x1624 = stage_load(NG - 1)
    la, xa = stage_load(0)
    mm24 = stage_l1(xts24)
    hh = stage_h1(mm24)
    mm2 = stage_l2(hh)
    h2 = stage_h2(mm2)
    o32 = stage_out(NG - 1, h2, x1624, want_o32=True)
    yt = chain_init(o32)

    NCH = KPRED - 1  # 19 chain steps
    ch_done = 0
    flushed = 0
    pairs = [(2 * i, 2 * i + 1) for i in range(12)]  # groups 0..23
    loads = {0: (la, xa)}

    def flush_chain(upto):
        nonlocal flushed
        nc.gpsimd.dma_start(
            t_outch[flushed:upto].rearrange("k p a b -> p k a b"),
            chainbuf[:, flushed:upto, :, :])
        flushed = upto

    for pi_, (ga, gb) in enumerate(pairs):
        if gb not in loads:
            loads[gb] = stage_load(gb)
        # prefetch next pair's inputs
        if pi_ + 1 < len(pairs):
            for ng in pairs[pi_ + 1]:
                if ng not in loads:
                    loads[ng] = stage_load(ng)
        mma = stage_l1(loads[ga][0])
        mmb = stage_l1(loads[gb][0])
        h1a = stage_h1(mma)
        h1b = stage_h1(mmb)
        mm2a = stage_l2(h1a)
        mm2b = stage_l2(h1b)
        h2a = stage_h2(mm2a)
        h2b = stage_h2(mm2b)
        stage_out(ga, h2a, loads[ga][1])
        stage_out(gb, h2b, loads[gb][1])
        del loads[ga], loads[gb]
        # 2 chain steps per pair slot until the 19 are done
        for _ in range(2):
            if ch_done < NCH:
                yt = chain_step(ch_done, yt)
                ch_done += 1
        if ch_done in (6, 12, NCH) and flushed < ch_done:
            flush_chain(ch_done)
    if flushed < NCH:
        flush_chain(NCH)


def _build(h):
    import concourse.mybir as mybir
    import concourse.tile as tile
    from concourse import bacc

    F32 = mybir.dt.float32
    BF16 = mybir.dt.bfloat16
    FP8 = mybir.dt.float8e4

    nc = bacc.Bacc("TRN2", target_bir_lowering=False, debug=False,
                   num_devices=NCORES)
    t_lat8u = nc.dram_tensor("lat8u", [PB, T_OBS, P], BF16,
                             kind="ExternalInput").ap()
    t_latb = nc.dram_tensor("latb", [PB, T_OBS, D], BF16,
                            kind="ExternalInput").ap()
    t_w1i = nc.dram_tensor("w1i", [P, 2, D], FP8, kind="ExternalInput").ap()
    t_w2i = nc.dram_tensor("w2i", [P, 2, D], FP8, kind="ExternalInput").ap()
    t_w3m = nc.dram_tensor("w3m", [P, 2, D], FP8, kind="ExternalInput").ap()
    t_w1c = nc.dram_tensor("w1c", [P, 2, D], FP8, kind="ExternalInput").ap()
    t_w3c = nc.dram_tensor("w3c", [P, 2, D], BF16, kind="ExternalInput").ap()
    t_hb3 = nc.dram_tensor("hb3", [1, D], BF16, kind="ExternalInput").ap()
    t_ones = nc.dram_tensor("ones", [1, P], BF16, kind="ExternalInput").ap()
    t_bs = nc.dram_tensor("bs", [P, 2], F32, kind="ExternalInput").ap()
    t_id32 = nc.dram_tensor("id32", [P, P], F32, kind="ExternalInput").ap()
    t_out16 = nc.dram_tensor("out16", [PB, T_OBS, D], BF16,
                             kind="ExternalOutput").ap()
    t_outch = nc.dram_tensor("outch", [KPRED - 1, P, 2, P], F32,
                             kind="ExternalOutput").ap()

    with tile.TileContext(nc) as tc, ExitStack() as ctx:
        _emit(ctx, tc, t_lat8u, t_latb, t_w1i, t_w2i, t_w3m, t_w1c, t_w3c,
              t_hb3, t_ones, t_bs, t_id32, t_out16, t_outch, h)
    nc.compile()
    return nc


def _host_inputs(inputs):
    ts = np.asarray(inputs["time_steps"], np.float32)
    h = float(np.float32(ts[1]) - np.float32(ts[0]))

    bf = ml_dtypes.bfloat16
    f8 = ml_dtypes.float8_e4m3

    W1 = np.asarray(inputs["W1"], np.float32)
    W2 = np.asarray(inputs["W2"], np.float32)
    W3 = np.asarray(inputs["W3"], np.float32)
    b1 = np.asarray(inputs["b1"], np.float32)
    b2 = np.asarray(inputs["b2"], np.float32)
    b3 = np.asarray(inputs["b3"], np.float32)

    # pairing permutations: sort U features by bias so partition-paired
    # features share (nearly) one bias value
    pi = np.argsort(b1, kind="stable")      # L1 outputs
    sig = np.argsort(b2, kind="stable")     # L2 outputs
    # column placement: feature at output slot (mc*128 + p) is perm[2p + mc]
    pi_col = np.empty(D, np.int64)
    sig_col = np.empty(D, np.int64)
    pp = np.arange(P)
    for mc in range(2):
        pi_col[mc * P + pp] = pi[2 * pp + mc]
        sig_col[mc * P + pp] = sig[2 * pp + mc]
    b1s = 0.5 * (b1[pi[0::2]] + b1[pi[1::2]])   # [128]
    b2s = 0.5 * (b2[sig[0::2]] + b2[sig[1::2]])
    bs = np.stack([b1s, b2s], axis=1).astype(np.float32)

    # w1i[p, j, n] = 8*W1[2p+j, pi_col[n]]   (interleaved k for dma-transpose)
    w1i = (8.0 * W1[:, pi_col]).astype(f8).reshape(P, 2, D)
    # w1c[p, j, n] = 8*W1[j*128+p, pi_col[n]]  (chunked k for the chain)
    w1c = np.ascontiguousarray(
        (8.0 * W1[:, pi_col]).astype(f8).reshape(2, P, D).transpose(1, 0, 2))
    # w2i[p, j, n] = 8*W2[pi_col[j*128+p], sig_col[n]]
    w2p = (8.0 * W2[pi_col][:, sig_col]).astype(f8)
    w2i = np.ascontiguousarray(w2p.reshape(2, P, D).transpose(1, 0, 2))
    # w3m[p, j, m] = 8*W3[sig_col[j*128+p], m]
    w3p = (8.0 * W3[sig_col]).astype(f8)
    w3m = np.ascontiguousarray(w3p.reshape(2, P, D).transpose(1, 0, 2))
    # w3c[p, kc, m] = h*W3[sig_col[kc*128+p], m]  bf16 for the chain
    w3c = np.ascontiguousarray(
        (np.float32(h) * W3[sig_col]).astype(bf).reshape(2, P, D)
        .transpose(1, 0, 2))
    hb3 = (np.float32(h) * b3).astype(bf).reshape(1, D)
    ones = np.ones((1, P), np.float32).astype(bf)
    id32 = np.eye(P, dtype=np.float32)

    shared = dict(w1i=w1i, w2i=w2i, w3m=w3m, w1c=w1c, w3c=w3c, hb3=hb3,
                  ones=ones, bs=bs, id32=id32)
    return h, shared


_CACHE = {}


def kernel(**inputs):
    from concourse.bass_utils import run_bass_kernel_spmd

    lat = np.ascontiguousarray(np.asarray(inputs["latents"], np.float32))
    h, shared = _host_inputs(inputs)
    b3 = np.asarray(inputs["b3"], np.float32)

    bf = ml_dtypes.bfloat16
    f8 = ml_dtypes.float8_e4m3
    lat8u = lat.astype(f8).view(np.uint16).view(bf)       # [B, 100, 128]
    latb = (lat + np.float32(h) * b3).astype(bf)          # [B, 100, 256]

    if h not in _CACHE:
        _CACHE[h] = _build(h)
    nc = _CACHE[h]

    in_maps = []
    for c in range(NCORES):
        m = dict(shared)
        m["lat8u"] = np.ascontiguousarray(lat8u[c * PB:(c + 1) * PB])
        m["latb"] = np.ascontiguousarray(latb[c * PB:(c + 1) * PB])
        in_maps.append(m)
    res = run_bass_kernel_spmd(nc, in_maps, list(range(NCORES)))

    out = np.empty((B, T, D), np.float32)
    out[:, 0] = lat[:, 0]
    for c in range(NCORES):
        sl = slice(c * PB, (c + 1) * PB)
        out[sl, 1:T_OBS + 1] = res.results[c]["out16"].astype(np.float32)
        ch = res.results[c]["outch"]          # [19, p, dc, b]
        out[sl, T_OBS + 1:] = ch.transpose(0, 3, 2, 1).reshape(
            KPRED - 1, P, D).transpose(1, 0, 2)
    out[:, 2] = lat[:, 1]
    return out
